# Initial kernel scaffold
#
"""DGCNN regression kernel for Trainium2, 8 NeuronCores, batch-parallel.

Each core processes 2 of the 16 samples (packed on SBUF partitions as
[sampleA(64ch); sampleB(64ch)]).  Pipeline per edge-conv layer:
  - augmented matmul produces the pairwise -sq-distance matrix pd in PSUM
  - DVE max8/max_index/match_replace extracts top-24 (keep 20) neighbor ids
  - an SBUF->SBUF permute DMA rewraps the ids into the indirect_copy stream
    layout (rank-major, wrapped per 16 partitions)
  - gpsimd indirect_copy gathers U = s*(w_nbr @ f) columns; DVE adds
    V = s*((w_ctr - w_nbr) @ f) + t with a broadcast AP; ACT applies LeakyReLU
  - second 1x1 conv (block-diagonal 2-sample matmul) + fused BN+LeakyReLU on
    PSUM eviction; DVE windowed tensor_reduce max-pools over the 20 neighbors
Head: w6 -> global max (per-m-tile reduce), w7 folded as (w7x @ [x1;x2;x3] +
bias(w7g @ g + t7)), w8, w9 + sigmoid.
"""

import numpy as np

import concourse.bass as bass
import concourse.mybir as mybir
import concourse.tile as tile_mod
from concourse.tile import TileContext
from concourse.vector_clock import ScopedClock, VectorClock
from concourse.bass_utils import run_bass_kernel_spmd

F32 = mybir.dt.float32
F32R = mybir.dt.float32r
U16 = mybir.dt.uint16
AF = mybir.ActivationFunctionType
ALU = mybir.AluOpType

N = 2048
KNN = 20
KSEL = 24  # extracted per row (3 x max8), first 20 used
NEG = -3.0e38


def _patched_drain_and_barrier(self, tick_clock, wait_clock):
    # This walrus build rejects >2 sem waits on a Drain; split them 1/drain.
    gc = tick_clock.global_clock
    for proc in range(len(gc)):
        t = gc[proc]
        if t > 0:
            vc = VectorClock()
            vc.require_at_least(proc, t)
            d = self.nc.sync.drain()
            wait_clock.add_sem_waits(d.ins, ScopedClock({None: vc}))
    self.nc.all_engine_barrier()
    assert self.sems is not None
    popped = self.nc._tile_sem_poison_stack.pop()
    assert popped is self._sem_poison
    self.nc.clear_and_free_semaphores(list(self.sems.allocated().values()))
    self.nc.all_engine_barrier()


TileContext._drain_and_barrier = _patched_drain_and_barrier

MAX_WAITS = 1


def split_sem_waits(nc, limit=MAX_WAITS):
    """This walrus build rejects instructions with more than ~2 sem waits.
    Move excess waits onto same-engine NoOps inserted just before."""
    for fn in nc.m.functions:
        for bb in fn.blocks:
            old = list(bb.instructions)
            new = []
            for inst in old:
                si = inst.sync_info
                waits = list(si.on_wait) if si is not None and si.on_wait else []
                ilim = limit
                if len(waits) > ilim:
                    extra, keep = waits[:-ilim], waits[-ilim:]
                    for i in range(0, len(extra), limit):
                        nop = mybir.InstNoOp(
                            name=nc.get_next_instruction_name(),
                            sync_info=mybir.SyncInfo(
                                on_wait=extra[i : i + limit], on_update=[]),
                            bass_nofuse=True,
                            engine=inst.engine,
                        )
                        new.append(nop)
                    si.on_wait = keep
                new.append(inst)
            if len(new) != len(old):
                try:
                    bb.instructions = new
                except Exception:
                    bb.instructions.clear()
                    for inst in new:
                        bb.add_instruction(inst)
    return nc


def _blockdiag(a, b):
    o = np.zeros((a.shape[0] + b.shape[0], a.shape[1] + b.shape[1]), np.float32)
    o[: a.shape[0], : a.shape[1]] = a
    o[a.shape[0] :, a.shape[1] :] = b
    return o


def host_preprocess(inp):
    """Fold BN into conv weights; build block-diagonal 2-sample matmul weights."""
    w = {}
    for li, (wk, sk, tk, cin) in enumerate(
        [("w1", "s1", "t1", 2), ("w3", "s3", "t3", 64), ("w5", "s5", "t5", 64)], 1
    ):
        W, s, t = inp[wk], inp[sk], inp[tk]
        wa, wb = W[:, :cin], W[:, cin:]
        lu = (wa * s[:, None]).T.astype(np.float32)          # [C, 64]
        lv = ((wb - wa) * s[:, None]).T.astype(np.float32)   # [C, 64]
        w[f"w_u{li}"] = _blockdiag(lu, lu)
        w[f"w_v{li}"] = _blockdiag(lv, lv)
        w[f"tv{li}"] = np.concatenate([t, t]).astype(np.float32)[:, None]  # [128,1]
    for ci, (wk, sk, tk) in enumerate([("w2", "s2", "t2"), ("w4", "s4", "t4")], 1):
        W = inp[wk].T.astype(np.float32)  # lhsT [64, 64]
        w[f"w_c{ci}"] = _blockdiag(W, W)
        w[f"sc{ci}"] = np.concatenate([inp[sk], inp[sk]]).astype(np.float32)[:, None]
        w[f"tc{ci}"] = np.concatenate([inp[tk], inp[tk]]).astype(np.float32)[:, None]
    w6t = inp["w6"].T.astype(np.float32)  # [192, 1024]
    w["w6a"] = w6t[:128]
    w["w6b"] = np.ascontiguousarray(w6t[128:])                     # [64, 1024]
    w["s6"] = np.ascontiguousarray(inp["s6"].reshape(8, 128).T)    # [128, 8]
    w["t6"] = np.ascontiguousarray(inp["t6"].reshape(8, 128).T)
    w7 = inp["w7"].astype(np.float32)
    w7g = w7[:, :1024].T  # [1024, 512]
    # kc-major: [128, 8*512]; slice (kc, mt) = [:, kc*512+mt*128 : +128]
    w["w7g"] = np.ascontiguousarray(
        w7g.reshape(8, 128, 512).transpose(1, 0, 2).reshape(128, 8 * 512)
    )
    w7xt = w7[:, 1024:].T  # [192, 512]
    w["w7xa"] = np.ascontiguousarray(w7xt[:128])
    w["w7xb"] = np.ascontiguousarray(w7xt[128:])
    w["s7"] = np.ascontiguousarray(inp["s7"].reshape(4, 128).T)
    w["t7"] = np.ascontiguousarray(inp["t7"].reshape(4, 128).T)
    w8t = inp["w8"].T.astype(np.float32)  # [512, 256]
    w["w8t"] = np.ascontiguousarray(
        w8t.reshape(4, 128, 256).transpose(1, 0, 2).reshape(128, 4 * 256)
    )
    w["s8"] = np.ascontiguousarray(inp["s8"].reshape(2, 128).T)
    w["t8"] = np.ascontiguousarray(inp["t8"].reshape(2, 128).T)
    w["w9t"] = np.ascontiguousarray(
        inp["w9"].astype(np.float32).reshape(2, 128).T  # [128, 2], col kc
    )
    w["onesblk"] = _blockdiag(np.ones((64, 1), np.float32), np.ones((64, 1), np.float32))
    w["onesblk1"] = _blockdiag(np.ones((2, 1), np.float32), np.ones((2, 1), np.float32))
    return w


def r32(ap):
    # v1: plain fp32 matmuls (fp32r requires producer-side rounding and its
    # ~11-bit mantissa is too coarse for kNN distance ranking).
    return ap


def build_nc(stage=4, debug=False):
    """stage: 1..3 = stop after edge layer `stage` (output x{stage} packed);
    4 = full network."""
    nc = bass.Bass("TRN2", target_bir_lowering=False)

    xpk = nc.dram_tensor("xpk", [4, N], F32, kind="ExternalInput")
    wts = {}

    def win(name, shape):
        wts[name] = nc.dram_tensor(name, shape, F32, kind="ExternalInput")

    for li, c in [(1, 2), (2, 64), (3, 64)]:
        win(f"w_u{li}", [2 * c, 128])
        win(f"w_v{li}", [2 * c, 128])
        win(f"tv{li}", [128, 1])
    for ci in (1, 2):
        win(f"w_c{ci}", [128, 128])
        win(f"sc{ci}", [128, 1])
        win(f"tc{ci}", [128, 1])
    win("onesblk", [128, 2])
    win("onesblk1", [4, 2])
    if stage >= 4:
        win("w6a", [128, 1024]); win("w6b", [64, 1024])
        win("s6", [128, 8]); win("t6", [128, 8])
        win("w7g", [128, 8 * 512]); win("w7xa", [128, 512]); win("w7xb", [64, 512])
        win("s7", [128, 4]); win("t7", [128, 4])
        win("w8t", [128, 4 * 256]); win("s8", [128, 2]); win("t8", [128, 2])
        win("w9t", [128, 2])

    if stage >= 4:
        y_out = nc.dram_tensor("y", [2, N], F32, kind="ExternalOutput")
    else:
        y_out = nc.dram_tensor("y", [128, N], F32, kind="ExternalOutput")
    dbg = {}
    if debug:
        for nm, shp, dt in [
            ("dbg_pd", [128, N], F32),
            ("dbg_idx", [128, KSEL], U16),
            ("dbg_u", [128, N], F32),
            ("dbg_v", [128, N], F32),
            ("dbg_g", [128, KNN * 128], F32),
            ("dbg_h", [128, KNN * 128], F32),
            ("dbg_h2", [128, KNN * 128], F32),
        ]:
            dbg[nm] = nc.dram_tensor(nm, shp, dt, kind="ExternalOutput")

    with TileContext(nc) as tc:
        cpool_cm = tc.tile_pool(name="consts", bufs=1)
        cpool = cpool_cm.__enter__()
        HEAD_W = ("w6a", "w6b", "s6", "t6", "w7g", "w7xa", "w7xb", "s7", "t7",
                  "w8t", "s8", "t8", "w9t")
        W = {k: cpool.tile_from(v[:], name=k) for k, v in wts.items()
             if k not in HEAD_W}

        # feature tensors (packed [A;B])
        feat = cpool.tile([4, N], F32)
        nc.sync.dma_start(out=feat, in_=xpk[:])
        onesP = cpool.tile([1, N], F32)
        onesM = cpool.tile([1, N], F32)
        nc.vector.memset(onesP, 1.0)
        nc.vector.memset(onesM, -1.0)
        xs = {}  # layer outputs x1,x2,x3 packed [128, N]

        def edge_layer(li, f, c, conv2):
            """f: packed [2c, N]; returns packed [128, N] pooled output."""
            pf = f"L{li}"
            lay_cm = tc.tile_pool(name=pf, bufs=1)
            lay = lay_cm.__enter__()
            uv_ps_cm = tc.tile_pool(name=pf + "uv", bufs=2, space="PSUM")
            uv_ps = uv_ps_cm.__enter__()

            # U, V [128, N]
            u_sb = lay.tile([128, N], F32, tag="u")
            v_sb = lay.tile([128, N], F32, tag="v")
            for dst, wname, bias in ((u_sb, f"w_u{li}", None), (v_sb, f"w_v{li}", W[f"tv{li}"])):
                for nchunk in range(N // 512):
                    ps = uv_ps.tile([128, 512], F32, tag="uvps")
                    sl = bass.ts(nchunk, 512)
                    nc.tensor.matmul(out=ps, lhsT=r32(W[wname][:, :]), rhs=r32(f[:, sl]),
                                     start=True, stop=True)
                    if bias is None:
                        nc.scalar.activation(out=dst[:, sl], in_=ps, func=AF.Copy)
                    else:
                        nc.scalar.activation(out=dst[:, sl], in_=ps, func=AF.Identity,
                                             bias=bias[:, 0:1])

            if debug and li == 1:
                nc.sync.dma_start(out=dbg["dbg_u"][:], in_=u_sb)
                nc.sync.dma_start(out=dbg["dbg_v"][:], in_=v_sb)

            # xx per sample: [2, N]
            sq = lay.tile([2 * c, N], F32, tag="sq")
            nc.vector.tensor_mul(sq, f, f)
            xx_sb = lay.tile([2, N], F32, tag="xx")
            xxn_sb = lay.tile([2, N], F32, tag="xxn")
            ones = W["onesblk1"] if li == 1 else W["onesblk"]
            for nchunk in range(N // 512):
                ps = uv_ps.tile([2, 512], F32, tag="xxps")
                sl = bass.ts(nchunk, 512)
                nc.tensor.matmul(out=ps, lhsT=r32(ones[:, :]), rhs=r32(sq[:, sl]),
                                 start=True, stop=True)
                nc.scalar.activation(out=xx_sb[:, sl], in_=ps, func=AF.Copy)
                nc.scalar.activation(out=xxn_sb[:, sl], in_=ps, func=AF.Copy, scale=-1.0)

            # augmented matrices per sample: L=[2f; xx; 1], R=[f; -xx; -1]
            augs = []
            for s in range(2):
                aL = lay.tile([c + 2, N], F32, tag=f"aL{s}")
                aR = lay.tile([c + 2, N], F32, tag=f"aR{s}")
                # pd[n,m] = sum_k L[k,n] R[k,m]:  L=[2f; xx; 1], R=[f; -1; -xx]
                nc.sync.dma_start(out=aR[0:c, :], in_=f[s * c : (s + 1) * c, :])
                nc.scalar.activation(out=aL[0:c, :], in_=aR[0:c, :], func=AF.Copy, scale=2.0)
                nc.sync.dma_start(out=aL[c : c + 1, :], in_=xx_sb[s : s + 1, :])
                nc.sync.dma_start(out=aR[c : c + 1, :], in_=onesM[:])
                nc.sync.dma_start(out=aL[c + 1 : c + 2, :], in_=onesP[:])
                nc.sync.dma_start(out=aR[c + 1 : c + 2, :], in_=xxn_sb[s : s + 1, :])
                augs.append((aL, aR))

            # top-k per sample -> g_all [128, 16*24] u16 + idx stream [128, 16*160]
            g_all = [lay.tile([128, 16 * KSEL], U16, tag=f"g{s}", name=f"g_all{s}")
                     for s in range(2)]
            stream = lay.tile([128, 16 * 8 * KNN], U16, tag="stream")
            pd_ps_cm = tc.tile_pool(name=pf + "pd", bufs=2, space="PSUM")
            pd_ps = pd_ps_cm.__enter__()
            tk_sb_cm = tc.tile_pool(name=pf + "tk", bufs=2)
            tk_sb = tk_sb_cm.__enter__()
            for s in range(2):
                aL, aR = augs[s]
                for u in range(16):
                    p_sb = tk_sb.tile([128, N], F32, tag="p")
                    for half in range(2):
                        ps = pd_ps.tile([128, 1024], F32, tag="pdps")
                        for q in range(2):
                            nc.tensor.matmul(
                                out=ps[:, q * 512 : (q + 1) * 512],
                                lhsT=r32(aL[:, u * 128 : (u + 1) * 128]),
                                rhs=r32(aR[:, half * 1024 + q * 512 : half * 1024 + (q + 1) * 512]),
                                start=True, stop=True)
                        nc.scalar.activation(
                            out=p_sb[:, half * 1024 : (half + 1) * 1024], in_=ps, func=AF.Copy)
                    if debug and s == 0 and u == 0 and li == 1:
                        nc.sync.dma_start(out=dbg["dbg_pd"][:], in_=p_sb)
                    m8 = tk_sb.tile([128, 8], F32, tag="m8")
                    gsl = g_all[s][:, u * KSEL : (u + 1) * KSEL]
                    for it in range(3):
                        nc.vector.max(out=m8, in_=p_sb)
                        nc.vector.max_index(
                            out=gsl[:, it * 8 : (it + 1) * 8], in_max=m8, in_values=p_sb)
                        if it < 2:
                            nc.vector.match_replace(
                                out=p_sb, in_to_replace=m8, in_values=p_sb, imm_value=NEG)
                if debug and s == 0 and li == 1:
                    nc.sync.dma_start(out=dbg["dbg_idx"][:], in_=g_all[0][:, 0:KSEL])

                # permute DMAs: g_all [128,(16u,24)] -> stream rows [16, 16u*160]
                # stream layout: idxs[r, u*160 + q*20 + c] = g[16q+r, u, c]
                # => gather col i (i=16*(20q+c)+r) is (point 16q+r, rank c)
                gv3 = g_all[s].rearrange("p (u c) -> p u c", c=KSEL)[:, :, 0:KNN]
                dstbase = 64 * s
                dv4 = stream.rearrange("p (u q1 c) -> p u q1 c", q1=8, c=KNN)
                for q in range(8):
                    nc.sync.dma_start(
                        out=dv4[dstbase : dstbase + 16, :, q, :],
                        in_=gv3[16 * q : 16 * (q + 1), :, :])
                for rep in range(1, 4):
                    nc.sync.dma_start(
                        out=stream[dstbase + 16 * rep : dstbase + 16 * (rep + 1), :],
                        in_=stream[dstbase : dstbase + 16, :])
            pd_ps_cm.__exit__(None, None, None)

            # gather + edge conv + pool
            xo = cpool.tile([128, N], F32, tag=f"x{li}")
            ed_sb_cm = tc.tile_pool(name=pf + "ed", bufs=2)
            ed_sb = ed_sb_cm.__enter__()
            c2_ps_cm = tc.tile_pool(name=pf + "c2", bufs=4, space="PSUM")
            c2_ps = c2_ps_cm.__enter__()
            for u in range(16):
                g_t = ed_sb.tile([128, KNN * 128], F32, tag="g")
                for sub in range(5):  # indirect_copy caps at 1024 indices
                    nc.gpsimd.indirect_copy(
                        out=g_t[:, sub * 512 : (sub + 1) * 512], data=u_sb,
                        idxs=stream[:, u * 160 + sub * 32 : u * 160 + (sub + 1) * 32],
                        i_know_ap_gather_is_preferred=True)
                if debug and u == 0 and li == 1:
                    nc.sync.dma_start(out=dbg["dbg_g"][:], in_=g_t)
                h_t = g_t
                vb = (v_sb[:, u * 128 : (u + 1) * 128]
                      .rearrange("o (q r) -> o q r", r=16)
                      .unsqueeze(2).to_broadcast([128, 8, KNN, 16]))
                nc.vector.tensor_tensor(
                    out=h_t.rearrange("o (q c r) -> o q c r", q=8, r=16),
                    in0=g_t.rearrange("o (q c r) -> o q c r", q=8, r=16),
                    in1=vb, op=ALU.add)
                nc.scalar.activation(out=h_t, in_=h_t, func=AF.Prelu, alpha=0.2)
                if debug and u == 0 and li == 1:
                    nc.sync.dma_start(out=dbg["dbg_h"][:], in_=h_t)
                if conv2 is not None:
                    wc, sc, tcb = conv2
                    h2_t = ed_sb.tile([128, KNN * 128], F32, tag="h2")
                    for ch in range(KNN * 128 // 512):
                        ps = c2_ps.tile([128, 512], F32, tag="c2ps")
                        sl = bass.ts(ch, 512)
                        nc.tensor.matmul(out=ps, lhsT=r32(wc[:, :]), rhs=r32(h_t[:, sl]),
                                         start=True, stop=True)
                        nc.scalar.activation(out=h2_t[:, sl], in_=ps, func=AF.Prelu,
                                             alpha=0.2, scale=sc[:, 0:1], bias=tcb[:, 0:1])
                    if debug and u == 0 and li == 1:
                        nc.sync.dma_start(out=dbg["dbg_h2"][:], in_=h2_t)
                    pool_in = h2_t
                else:
                    pool_in = h_t
                nc.vector.tensor_reduce(
                    out=xo[:, u * 128 : (u + 1) * 128],
                    in_=pool_in.rearrange("o (q c r) -> o q r c", q=8, c=KNN),
                    axis=mybir.AxisListType.X, op=ALU.max)
            c2_ps_cm.__exit__(None, None, None)
            ed_sb_cm.__exit__(None, None, None)
            tk_sb_cm.__exit__(None, None, None)
            uv_ps_cm.__exit__(None, None, None)
            lay_cm.__exit__(None, None, None)
            return xo

        x1 = edge_layer(1, feat, 2, (W["w_c1"], W["sc1"], W["tc1"]))
        xs["x1"] = x1
        if stage == 1:
            nc.sync.dma_start(out=y_out[:], in_=x1)
        if stage >= 2:
            x2 = edge_layer(2, x1, 64, (W["w_c2"], W["sc2"], W["tc2"]))
            xs["x2"] = x2
            if stage == 2:
                nc.sync.dma_start(out=y_out[:], in_=x2)
        if stage >= 3:
            x3 = edge_layer(3, x2, 64, None)
            xs["x3"] = x3
            if stage == 3:
                nc.sync.dma_start(out=y_out[:], in_=x3)

        if stage >= 4:
            hd_cm = tc.tile_pool(name="head", bufs=1)
            hd = hd_cm.__enter__()
            for k in HEAD_W:
                W[k] = hd.tile_from(wts[k][:], name=k)
            hd_ps_cm = tc.tile_pool(name="headps", bufs=4, space="PSUM")
            hd_ps = hd_ps_cm.__enter__()
            hs_cm = tc.tile_pool(name="headsb", bufs=3)
            hs = hs_cm.__enter__()
            for s in range(2):
                # cat = [x1_s; x2_s] [128, N], x3_s [64, N]
                cat = hd.tile([128, N], F32, tag="cat")
                x3s = hd.tile([64, N], F32, tag="x3s")
                nc.sync.dma_start(out=cat[0:64, :], in_=x1[s * 64 : (s + 1) * 64, :])
                nc.sync.dma_start(out=cat[64:128, :], in_=x2[s * 64 : (s + 1) * 64, :])
                nc.sync.dma_start(out=x3s[:], in_=x3[s * 64 : (s + 1) * 64, :])

                # w6 + global max -> gmax [128, 8]
                gmax = hd.tile([128, 8], F32, tag="gmax")
                for m in range(8):
                    e_row = hs.tile([128, N], F32, tag="erow")
                    for nch in range(4):
                        ps = hd_ps.tile([128, 512], F32, tag="hps")
                        sl = bass.ts(nch, 512)
                        nc.tensor.matmul(out=ps, lhsT=r32(W["w6a"][:, m * 128 : (m + 1) * 128]),
                                         rhs=r32(cat[:, sl]), start=True, stop=False)
                        nc.tensor.matmul(out=ps, lhsT=r32(W["w6b"][:, m * 128 : (m + 1) * 128]),
                                         rhs=r32(x3s[:, sl]), start=False, stop=True)
                        nc.scalar.activation(out=e_row[:, sl], in_=ps, func=AF.Prelu,
                                             alpha=0.2, scale=W["s6"][:, m : m + 1],
                                             bias=W["t6"][:, m : m + 1])
                    nc.vector.tensor_reduce(out=gmax[:, m : m + 1], in_=e_row,
                                            axis=mybir.AxisListType.X, op=ALU.max)

                # bias7 = s7 * (w7g @ g) + t7  [128, 4]
                b7 = hd.tile([128, 4], F32, tag="b7")
                for mt in range(4):
                    ps = hd_ps.tile([128, 1], F32, tag="hps")
                    for kc in range(8):
                        nc.tensor.matmul(
                            out=ps,
                            lhsT=r32(W["w7g"][:, kc * 512 + mt * 128 : kc * 512 + (mt + 1) * 128]),
                            rhs=r32(gmax[:, kc : kc + 1]),
                            start=(kc == 0), stop=(kc == 7))
                    nc.scalar.activation(out=b7[:, mt : mt + 1], in_=ps, func=AF.Copy)
                nc.vector.tensor_mul(b7, b7, W["s7"])
                nc.vector.tensor_add(b7, b7, W["t7"])

                # h7 = Lrelu(s7*(w7x@cat2) + b7)  [128, 4*N]
                h7 = hd.tile([128, 4 * N], F32, tag="h7")
                for mt in range(4):
                    for nch in range(4):
                        ps = hd_ps.tile([128, 512], F32, tag="hps")
                        sl = bass.ts(nch, 512)
                        nc.tensor.matmul(out=ps, lhsT=r32(W["w7xa"][:, mt * 128 : (mt + 1) * 128]),
                                         rhs=r32(cat[:, sl]), start=True, stop=False)
                        nc.tensor.matmul(out=ps, lhsT=r32(W["w7xb"][:, mt * 128 : (mt + 1) * 128]),
                                         rhs=r32(x3s[:, sl]), start=False, stop=True)
                        nc.scalar.activation(out=h7[:, mt * N + nch * 512 : mt * N + (nch + 1) * 512],
                                             in_=ps, func=AF.Prelu, alpha=0.2,
                                             scale=W["s7"][:, mt : mt + 1],
                                             bias=b7[:, mt : mt + 1])

                # h8 = Lrelu(s8*(w8@h7) + t8) [128, 2*N]
                h8 = hd.tile([128, 2 * N], F32, tag="h8")
                for mt in range(2):
                    for nch in range(4):
                        ps = hd_ps.tile([128, 512], F32, tag="hps")
                        sl = bass.ts(nch, 512)
                        for kc in range(4):
                            nc.tensor.matmul(
                                out=ps,
                                lhsT=r32(W["w8t"][:, kc * 256 + mt * 128 : kc * 256 + (mt + 1) * 128]),
                                rhs=r32(h7[:, kc * N + nch * 512 : kc * N + (nch + 1) * 512]),
                                start=(kc == 0), stop=(kc == 3))
                        nc.scalar.activation(out=h8[:, mt * N + nch * 512 : mt * N + (nch + 1) * 512],
                                             in_=ps, func=AF.Prelu, alpha=0.2,
                                             scale=W["s8"][:, mt : mt + 1],
                                             bias=W["t8"][:, mt : mt + 1])

                # y = sigmoid(w9 @ h8) [1, N]
                yrow = hd.tile([1, N], F32, tag="yrow")
                for nch in range(4):
                    ps = hd_ps.tile([1, 512], F32, tag="hps")
                    sl = bass.ts(nch, 512)
                    for kc in range(2):
                        nc.tensor.matmul(
                            out=ps, lhsT=r32(W["w9t"][:, kc : kc + 1]),
                            rhs=r32(h8[:, kc * N + nch * 512 : kc * N + (nch + 1) * 512]),
                            start=(kc == 0), stop=(kc == 1))
                    nc.scalar.activation(out=yrow[:, sl], in_=ps, func=AF.Sigmoid)
                nc.sync.dma_start(out=y_out[s : s + 1, :], in_=yrow)
            hs_cm.__exit__(None, None, None)
            hd_ps_cm.__exit__(None, None, None)
            hd_cm.__exit__(None, None, None)
        cpool_cm.__exit__(None, None, None)
    split_sem_waits(nc)
    return nc


_NC_CACHE = {}
LAST_RESULT = None  # BassKernelResults of the most recent kernel() call


def kernel(**inputs):
    global LAST_RESULT
    stage = inputs.pop("_stage", 4)
    debug = inputs.pop("_debug", False)
    trace = inputs.pop("_trace", False)
    key = (stage, debug)
    if key not in _NC_CACHE:
        _NC_CACHE[key] = build_nc(stage, debug)
    nc = _NC_CACHE[key]

    w = host_preprocess(inputs)
    x = np.asarray(inputs["x"], np.float32)  # [16, 2, 2048]
    in_maps = []
    for core in range(8):
        m = dict(w) if stage >= 4 else {
            k: v for k, v in w.items()
            if not k.startswith(("w6", "w7", "w8", "w9", "s6", "s7", "s8", "t6", "t7", "t8"))
        }
        m["xpk"] = np.concatenate([x[2 * core], x[2 * core + 1]], 0)  # [4, N]
        in_maps.append(m)
    res = run_bass_kernel_spmd(nc, in_maps, core_ids=list(range(8)), trace=trace)
    LAST_RESULT = res
    outs = [res.results[i]["y"] for i in range(8)]
    if stage >= 4:
        return np.concatenate(outs, 0).astype(np.float32)  # [16, N]
    return np.stack(outs, 0)  # debug: [8, 128, N]



# revision 14
# speedup vs baseline: 1.3219x; 1.3219x over previous
"""DGCNN regression kernel for Trainium2, 8 NeuronCores, batch-parallel.

Each core processes 2 of the 16 samples (packed on SBUF partitions as
[sampleA(64ch); sampleB(64ch)]).  Pipeline per edge-conv layer:
  - augmented matmul produces the pairwise -sq-distance matrix pd in PSUM
  - DVE max8/max_index/match_replace extracts top-24 (keep 20) neighbor ids
  - an SBUF->SBUF permute DMA rewraps the ids into the indirect_copy stream
    layout (rank-major, wrapped per 16 partitions)
  - gpsimd indirect_copy gathers U = s*(w_nbr @ f) columns; DVE adds
    V = s*((w_ctr - w_nbr) @ f) + t with a broadcast AP; ACT applies LeakyReLU
  - second 1x1 conv (block-diagonal 2-sample matmul) + fused BN+LeakyReLU on
    PSUM eviction; DVE windowed tensor_reduce max-pools over the 20 neighbors
Head: w6 -> global max (per-m-tile reduce), w7 folded as (w7x @ [x1;x2;x3] +
bias(w7g @ g + t7)), w8, w9 + sigmoid.
"""

import numpy as np

import concourse.bass as bass
import concourse.mybir as mybir
import concourse.tile as tile_mod
from concourse.tile import TileContext
from concourse.vector_clock import ScopedClock, VectorClock
from concourse.bass_utils import run_bass_kernel_spmd

F32 = mybir.dt.float32
F32R = mybir.dt.float32r
U16 = mybir.dt.uint16
AF = mybir.ActivationFunctionType
ALU = mybir.AluOpType

N = 2048
KNN = 20
KSEL = 24  # extracted per row (3 x max8), first 20 used
NEG = -3.0e38


def _patched_drain_and_barrier(self, tick_clock, wait_clock):
    # This walrus build rejects >2 sem waits on a Drain; split them 1/drain.
    gc = tick_clock.global_clock
    for proc in range(len(gc)):
        t = gc[proc]
        if t > 0:
            vc = VectorClock()
            vc.require_at_least(proc, t)
            d = self.nc.sync.drain()
            wait_clock.add_sem_waits(d.ins, ScopedClock({None: vc}))
    self.nc.all_engine_barrier()
    assert self.sems is not None
    popped = self.nc._tile_sem_poison_stack.pop()
    assert popped is self._sem_poison
    self.nc.clear_and_free_semaphores(list(self.sems.allocated().values()))
    self.nc.all_engine_barrier()


TileContext._drain_and_barrier = _patched_drain_and_barrier

MAX_WAITS = 1


def split_sem_waits(nc, limit=MAX_WAITS):
    """This walrus build rejects instructions with more than ~2 sem waits.
    Move excess waits onto same-engine NoOps inserted just before."""
    for fn in nc.m.functions:
        for bb in fn.blocks:
            old = list(bb.instructions)
            new = []
            for inst in old:
                si = inst.sync_info
                waits = list(si.on_wait) if si is not None and si.on_wait else []
                ilim = limit
                if len(waits) > ilim:
                    extra, keep = waits[:-ilim], waits[-ilim:]
                    for i in range(0, len(extra), limit):
                        nop = mybir.InstNoOp(
                            name=nc.get_next_instruction_name(),
                            sync_info=mybir.SyncInfo(
                                on_wait=extra[i : i + limit], on_update=[]),
                            bass_nofuse=True,
                            engine=inst.engine,
                        )
                        new.append(nop)
                    si.on_wait = keep
                new.append(inst)
            if len(new) != len(old):
                try:
                    bb.instructions = new
                except Exception:
                    bb.instructions.clear()
                    for inst in new:
                        bb.add_instruction(inst)
    return nc


def _blockdiag(a, b):
    o = np.zeros((a.shape[0] + b.shape[0], a.shape[1] + b.shape[1]), np.float32)
    o[: a.shape[0], : a.shape[1]] = a
    o[a.shape[0] :, a.shape[1] :] = b
    return o


def host_preprocess(inp):
    """Fold BN into conv weights; build block-diagonal 2-sample matmul weights."""
    w = {}
    for li, (wk, sk, tk, cin) in enumerate(
        [("w1", "s1", "t1", 2), ("w3", "s3", "t3", 64), ("w5", "s5", "t5", 64)], 1
    ):
        W, s, t = inp[wk], inp[sk], inp[tk]
        wa, wb = W[:, :cin], W[:, cin:]
        lu = (wa * s[:, None]).T.astype(np.float32)          # [C, 64]
        lv = ((wb - wa) * s[:, None]).T.astype(np.float32)   # [C, 64]
        w[f"w_u{li}"] = _blockdiag(lu, lu)
        w[f"w_v{li}"] = _blockdiag(lv, lv)
        w[f"tv{li}"] = np.concatenate([t, t]).astype(np.float32)[:, None]  # [128,1]
    for ci, (wk, sk, tk) in enumerate([("w2", "s2", "t2"), ("w4", "s4", "t4")], 1):
        W = inp[wk].T.astype(np.float32)  # lhsT [64, 64]
        w[f"w_c{ci}"] = _blockdiag(W, W)
        w[f"sc{ci}"] = np.concatenate([inp[sk], inp[sk]]).astype(np.float32)[:, None]
        w[f"tc{ci}"] = np.concatenate([inp[tk], inp[tk]]).astype(np.float32)[:, None]
    w6t = inp["w6"].T.astype(np.float32)  # [192, 1024]
    w["w6a"] = w6t[:128]
    w["w6b"] = np.ascontiguousarray(w6t[128:])                     # [64, 1024]
    w["s6"] = np.ascontiguousarray(inp["s6"].reshape(8, 128).T)    # [128, 8]
    w["t6"] = np.ascontiguousarray(inp["t6"].reshape(8, 128).T)
    w7 = inp["w7"].astype(np.float32)
    w7g = w7[:, :1024].T  # [1024, 512]
    # kc-major: [128, 8*512]; slice (kc, mt) = [:, kc*512+mt*128 : +128]
    w["w7g"] = np.ascontiguousarray(
        w7g.reshape(8, 128, 512).transpose(1, 0, 2).reshape(128, 8 * 512)
    )
    w7xt = w7[:, 1024:].T  # [192, 512]
    w["w7xa"] = np.ascontiguousarray(w7xt[:128])
    w["w7xb"] = np.ascontiguousarray(w7xt[128:])
    w["s7"] = np.ascontiguousarray(inp["s7"].reshape(4, 128).T)
    w["t7"] = np.ascontiguousarray(inp["t7"].reshape(4, 128).T)
    w8t = inp["w8"].T.astype(np.float32)  # [512, 256]
    w["w8t"] = np.ascontiguousarray(
        w8t.reshape(4, 128, 256).transpose(1, 0, 2).reshape(128, 4 * 256)
    )
    w["s8"] = np.ascontiguousarray(inp["s8"].reshape(2, 128).T)
    w["t8"] = np.ascontiguousarray(inp["t8"].reshape(2, 128).T)
    w["w9t"] = np.ascontiguousarray(
        inp["w9"].astype(np.float32).reshape(2, 128).T  # [128, 2], col kc
    )
    w["onesblk"] = _blockdiag(np.ones((64, 1), np.float32), np.ones((64, 1), np.float32))
    w["onesblk1"] = _blockdiag(np.ones((2, 1), np.float32), np.ones((2, 1), np.float32))
    return w


def r32(ap):
    # v1: plain fp32 matmuls (fp32r requires producer-side rounding and its
    # ~11-bit mantissa is too coarse for kNN distance ranking).
    return ap


def build_nc(stage=4, debug=False):
    """stage: 1..3 = stop after edge layer `stage` (output x{stage} packed);
    4 = full network."""
    nc = bass.Bass("TRN2", target_bir_lowering=False)

    xpk = nc.dram_tensor("xpk", [4, N], F32, kind="ExternalInput")
    wts = {}

    def win(name, shape):
        wts[name] = nc.dram_tensor(name, shape, F32, kind="ExternalInput")

    for li, c in [(1, 2), (2, 64), (3, 64)]:
        win(f"w_u{li}", [2 * c, 128])
        win(f"w_v{li}", [2 * c, 128])
        win(f"tv{li}", [128, 1])
    for ci in (1, 2):
        win(f"w_c{ci}", [128, 128])
        win(f"sc{ci}", [128, 1])
        win(f"tc{ci}", [128, 1])
    win("onesblk", [128, 2])
    win("onesblk1", [4, 2])
    if stage >= 4:
        win("w6a", [128, 1024]); win("w6b", [64, 1024])
        win("s6", [128, 8]); win("t6", [128, 8])
        win("w7g", [128, 8 * 512]); win("w7xa", [128, 512]); win("w7xb", [64, 512])
        win("s7", [128, 4]); win("t7", [128, 4])
        win("w8t", [128, 4 * 256]); win("s8", [128, 2]); win("t8", [128, 2])
        win("w9t", [128, 2])

    if stage >= 4:
        y_out = nc.dram_tensor("y", [2, N], F32, kind="ExternalOutput")
    else:
        y_out = nc.dram_tensor("y", [128, N], F32, kind="ExternalOutput")
    dbg = {}
    if debug:
        for nm, shp, dt in [
            ("dbg_pd", [128, N], F32),
            ("dbg_idx", [128, KSEL], U16),
            ("dbg_u", [128, N], F32),
            ("dbg_v", [128, N], F32),
            ("dbg_g", [128, KNN * 128], F32),
            ("dbg_h", [128, KNN * 128], F32),
            ("dbg_h2", [128, KNN * 128], F32),
        ]:
            dbg[nm] = nc.dram_tensor(nm, shp, dt, kind="ExternalOutput")

    with TileContext(nc) as tc:
        cpool_cm = tc.tile_pool(name="consts", bufs=1)
        cpool = cpool_cm.__enter__()
        HEAD_W = ("w6a", "w6b", "s6", "t6", "w7g", "w7xa", "w7xb", "s7", "t7",
                  "w8t", "s8", "t8", "w9t")
        W = {k: cpool.tile_from(v[:], name=k) for k, v in wts.items()
             if k not in HEAD_W}

        # feature tensors (packed [A;B])
        feat = cpool.tile([4, N], F32)
        nc.sync.dma_start(out=feat, in_=xpk[:])
        onesP = cpool.tile([1, N], F32)
        onesM = cpool.tile([1, N], F32)
        nc.vector.memset(onesP, 1.0)
        nc.vector.memset(onesM, -1.0)
        xs = {}  # layer outputs x1,x2,x3 packed [128, N]

        def edge_layer(li, f, c, conv2):
            """f: packed [2c, N]; returns packed [128, N] pooled output.

            Tile-major interleaved schedule: for each u-tile, both samples'
            pd/topk (DVE) run, the per-tile index stream is rewrapped, and the
            gather+conv+pool for that tile is issued immediately — so the
            gpsimd gather for tile u overlaps the DVE topk of tiles u+1...
            """
            pf = f"L{li}"
            lay_cm = tc.tile_pool(name=pf, bufs=1)
            lay = lay_cm.__enter__()
            uv_ps_cm = tc.tile_pool(name=pf + "uv", bufs=2, space="PSUM")
            uv_ps = uv_ps_cm.__enter__()

            # U, V [128, N]
            u_sb = lay.tile([128, N], F32, tag="u")
            v_sb = lay.tile([128, N], F32, tag="v")
            for dst, wname, bias in ((u_sb, f"w_u{li}", None), (v_sb, f"w_v{li}", W[f"tv{li}"])):
                for nchunk in range(N // 512):
                    ps = uv_ps.tile([128, 512], F32, tag="uvps")
                    sl = bass.ts(nchunk, 512)
                    nc.tensor.matmul(out=ps, lhsT=r32(W[wname][:, :]), rhs=r32(f[:, sl]),
                                     start=True, stop=True)
                    if bias is None:
                        nc.scalar.activation(out=dst[:, sl], in_=ps, func=AF.Copy)
                    else:
                        nc.scalar.activation(out=dst[:, sl], in_=ps, func=AF.Identity,
                                             bias=bias[:, 0:1])

            if debug and li == 1:
                nc.sync.dma_start(out=dbg["dbg_u"][:], in_=u_sb)
                nc.sync.dma_start(out=dbg["dbg_v"][:], in_=v_sb)

            # xx per sample: [2, N]
            sq = lay.tile([2 * c, N], F32, tag="sq")
            nc.vector.tensor_mul(sq, f, f)
            xx_sb = lay.tile([2, N], F32, tag="xx")
            xxn_sb = lay.tile([2, N], F32, tag="xxn")
            ones = W["onesblk1"] if li == 1 else W["onesblk"]
            for nchunk in range(N // 512):
                ps = uv_ps.tile([2, 512], F32, tag="xxps")
                sl = bass.ts(nchunk, 512)
                nc.tensor.matmul(out=ps, lhsT=r32(ones[:, :]), rhs=r32(sq[:, sl]),
                                 start=True, stop=True)
                nc.scalar.activation(out=xx_sb[:, sl], in_=ps, func=AF.Copy)
                nc.scalar.activation(out=xxn_sb[:, sl], in_=ps, func=AF.Copy, scale=-1.0)

            # augmented matrices per sample: L=[2f; xx; 1], R=[f; -xx; -1]
            augs = []
            for s in range(2):
                aL = lay.tile([c + 2, N], F32, tag=f"aL{s}")
                aR = lay.tile([c + 2, N], F32, tag=f"aR{s}")
                # pd[n,m] = sum_k L[k,n] R[k,m]:  L=[2f; xx; 1], R=[f; -1; -xx]
                nc.sync.dma_start(out=aR[0:c, :], in_=f[s * c : (s + 1) * c, :])
                nc.scalar.activation(out=aL[0:c, :], in_=aR[0:c, :], func=AF.Copy, scale=2.0)
                nc.sync.dma_start(out=aL[c : c + 1, :], in_=xx_sb[s : s + 1, :])
                nc.sync.dma_start(out=aR[c : c + 1, :], in_=onesM[:])
                nc.sync.dma_start(out=aL[c + 1 : c + 2, :], in_=onesP[:])
                nc.sync.dma_start(out=aR[c + 1 : c + 2, :], in_=xxn_sb[s : s + 1, :])
                augs.append((aL, aR))
            uv_ps_cm.__exit__(None, None, None)

            g_all = [lay.tile([128, 16 * KSEL], U16, tag=f"g{s}", name=f"g_all{s}")
                     for s in range(2)]
            stream = lay.tile([128, 16 * 8 * KNN], U16, tag="stream")
            xo = cpool.tile([128, N], F32, tag=f"x{li}")
            pd_ps_cm = tc.tile_pool(name=pf + "pd", bufs=2, space="PSUM")
            pd_ps = pd_ps_cm.__enter__()
            tk_sb_cm = tc.tile_pool(name=pf + "tk", bufs=2)
            tk_sb = tk_sb_cm.__enter__()
            ed_sb_cm = tc.tile_pool(name=pf + "ed", bufs=2)
            ed_sb = ed_sb_cm.__enter__()
            gt_sb_cm = tc.tile_pool(name=pf + "gt", bufs=3)
            gt_sb = gt_sb_cm.__enter__()
            c2_ps_cm = tc.tile_pool(name=pf + "c2", bufs=2, space="PSUM")
            c2_ps = c2_ps_cm.__enter__()

            gv3 = [g_all[s].rearrange("p (u c) -> p u c", c=KSEL)[:, :, 0:KNN]
                   for s in range(2)]
            dv4 = stream.rearrange("p (u q1 c) -> p u q1 c", q1=8, c=KNN)

            def topk_tile(s, u):
                aL, aR = augs[s]
                p_sb = tk_sb.tile([128, N], F32, tag="p")
                for half in range(2):
                    ps = pd_ps.tile([128, 1024], F32, tag="pdps")
                    for q in range(2):
                        nc.tensor.matmul(
                            out=ps[:, q * 512 : (q + 1) * 512],
                            lhsT=r32(aL[:, u * 128 : (u + 1) * 128]),
                            rhs=r32(aR[:, half * 1024 + q * 512 : half * 1024 + (q + 1) * 512]),
                            start=True, stop=True)
                    nc.scalar.activation(
                        out=p_sb[:, half * 1024 : (half + 1) * 1024], in_=ps, func=AF.Copy)
                if debug and s == 0 and u == 0 and li == 1:
                    nc.sync.dma_start(out=dbg["dbg_pd"][:], in_=p_sb)
                m8 = tk_sb.tile([128, 8], F32, tag="m8")
                gsl = g_all[s][:, u * KSEL : (u + 1) * KSEL]
                for it in range(3):
                    nc.vector.max(out=m8, in_=p_sb)
                    nc.vector.max_index(
                        out=gsl[:, it * 8 : (it + 1) * 8], in_max=m8, in_values=p_sb)
                    if it < 2:
                        nc.vector.match_replace(
                            out=p_sb, in_to_replace=m8, in_values=p_sb, imm_value=NEG)
                if debug and s == 0 and u == 0 and li == 1:
                    nc.sync.dma_start(out=dbg["dbg_idx"][:], in_=g_all[0][:, 0:KSEL])
                # per-tile permute DMAs into the gather stream:
                # stream[64s + r, u, q, c] = g_all[s][16q + r, u, c]
                dstbase = 64 * s
                for q in range(8):
                    nc.sync.dma_start(
                        out=dv4[dstbase : dstbase + 16, u : u + 1, q, :],
                        in_=gv3[s][16 * q : 16 * (q + 1), u : u + 1, :])
                for rep in range(1, 4):
                    nc.sync.dma_start(
                        out=dv4[dstbase + 16 * rep : dstbase + 16 * (rep + 1), u : u + 1, :, :],
                        in_=dv4[dstbase : dstbase + 16, u : u + 1, :, :])

            def conv_tile(u):
                g_t = gt_sb.tile([128, KNN * 128], F32, tag="g")
                for sub in range(5):  # indirect_copy caps at 1024 indices
                    nc.gpsimd.indirect_copy(
                        out=g_t[:, sub * 512 : (sub + 1) * 512], data=u_sb,
                        idxs=stream[:, u * 160 + sub * 32 : u * 160 + (sub + 1) * 32],
                        i_know_ap_gather_is_preferred=True)
                if debug and u == 0 and li == 1:
                    nc.sync.dma_start(out=dbg["dbg_g"][:], in_=g_t)
                h_t = g_t
                vb = (v_sb[:, u * 128 : (u + 1) * 128]
                      .rearrange("o (q r) -> o q r", r=16)
                      .unsqueeze(2).to_broadcast([128, 8, KNN, 16]))
                nc.vector.tensor_tensor(
                    out=h_t.rearrange("o (q c r) -> o q c r", q=8, r=16),
                    in0=g_t.rearrange("o (q c r) -> o q c r", q=8, r=16),
                    in1=vb, op=ALU.add)
                nc.scalar.activation(out=h_t, in_=h_t, func=AF.Prelu, alpha=0.2)
                if debug and u == 0 and li == 1:
                    nc.sync.dma_start(out=dbg["dbg_h"][:], in_=h_t)
                if conv2 is not None:
                    wc, sc, tcb = conv2
                    h2_t = ed_sb.tile([128, KNN * 128], F32, tag="h2")
                    for ch in range(KNN * 128 // 512):
                        ps = c2_ps.tile([128, 512], F32, tag="c2ps")
                        sl = bass.ts(ch, 512)
                        nc.tensor.matmul(out=ps, lhsT=r32(wc[:, :]), rhs=r32(h_t[:, sl]),
                                         start=True, stop=True)
                        nc.scalar.activation(out=h2_t[:, sl], in_=ps, func=AF.Prelu,
                                             alpha=0.2, scale=sc[:, 0:1], bias=tcb[:, 0:1])
                    if debug and u == 0 and li == 1:
                        nc.sync.dma_start(out=dbg["dbg_h2"][:], in_=h2_t)
                    pool_in = h2_t
                else:
                    pool_in = h_t
                nc.vector.tensor_reduce(
                    out=xo[:, u * 128 : (u + 1) * 128],
                    in_=pool_in.rearrange("o (q c r) -> o q r c", q=8, c=KNN),
                    axis=mybir.AxisListType.X, op=ALU.max)

            # Software pipelining: emit topk(u+1) BEFORE conv(u) so the DVE
            # (strict FIFO) runs tile u+1's topk while gpsimd gathers tile u —
            # otherwise conv(u)'s DVE add/pool would block topk(u+1) behind
            # the 71us gather and serialize the whole layer.
            LOOKAHEAD = 2
            for w in range(min(LOOKAHEAD, 16)):
                topk_tile(0, w)
                topk_tile(1, w)
            for u in range(16):
                if u + LOOKAHEAD < 16:
                    topk_tile(0, u + LOOKAHEAD)
                    topk_tile(1, u + LOOKAHEAD)
                conv_tile(u)

            c2_ps_cm.__exit__(None, None, None)
            gt_sb_cm.__exit__(None, None, None)
            ed_sb_cm.__exit__(None, None, None)
            tk_sb_cm.__exit__(None, None, None)
            pd_ps_cm.__exit__(None, None, None)
            lay_cm.__exit__(None, None, None)
            return xo

        x1 = edge_layer(1, feat, 2, (W["w_c1"], W["sc1"], W["tc1"]))
        xs["x1"] = x1
        if stage == 1:
            nc.sync.dma_start(out=y_out[:], in_=x1)
        if stage >= 2:
            x2 = edge_layer(2, x1, 64, (W["w_c2"], W["sc2"], W["tc2"]))
            xs["x2"] = x2
            if stage == 2:
                nc.sync.dma_start(out=y_out[:], in_=x2)
        if stage >= 3:
            x3 = edge_layer(3, x2, 64, None)
            xs["x3"] = x3
            if stage == 3:
                nc.sync.dma_start(out=y_out[:], in_=x3)

        if stage >= 4:
            hd_cm = tc.tile_pool(name="head", bufs=1)
            hd = hd_cm.__enter__()
            for k in HEAD_W:
                W[k] = hd.tile_from(wts[k][:], name=k)
            hd_ps_cm = tc.tile_pool(name="headps", bufs=4, space="PSUM")
            hd_ps = hd_ps_cm.__enter__()
            hs_cm = tc.tile_pool(name="headsb", bufs=3)
            hs = hs_cm.__enter__()
            for s in range(2):
                # cat = [x1_s; x2_s] [128, N], x3_s [64, N]
                cat = hd.tile([128, N], F32, tag="cat")
                x3s = hd.tile([64, N], F32, tag="x3s")
                nc.sync.dma_start(out=cat[0:64, :], in_=x1[s * 64 : (s + 1) * 64, :])
                nc.sync.dma_start(out=cat[64:128, :], in_=x2[s * 64 : (s + 1) * 64, :])
                nc.sync.dma_start(out=x3s[:], in_=x3[s * 64 : (s + 1) * 64, :])

                # w6 + global max -> gmax [128, 8]
                gmax = hd.tile([128, 8], F32, tag="gmax")
                for m in range(8):
                    e_row = hs.tile([128, N], F32, tag="erow")
                    for nch in range(4):
                        ps = hd_ps.tile([128, 512], F32, tag="hps")
                        sl = bass.ts(nch, 512)
                        nc.tensor.matmul(out=ps, lhsT=r32(W["w6a"][:, m * 128 : (m + 1) * 128]),
                                         rhs=r32(cat[:, sl]), start=True, stop=False)
                        nc.tensor.matmul(out=ps, lhsT=r32(W["w6b"][:, m * 128 : (m + 1) * 128]),
                                         rhs=r32(x3s[:, sl]), start=False, stop=True)
                        nc.scalar.activation(out=e_row[:, sl], in_=ps, func=AF.Prelu,
                                             alpha=0.2, scale=W["s6"][:, m : m + 1],
                                             bias=W["t6"][:, m : m + 1])
                    nc.vector.tensor_reduce(out=gmax[:, m : m + 1], in_=e_row,
                                            axis=mybir.AxisListType.X, op=ALU.max)

                # bias7 = s7 * (w7g @ g) + t7  [128, 4]
                b7 = hd.tile([128, 4], F32, tag="b7")
                for mt in range(4):
                    ps = hd_ps.tile([128, 1], F32, tag="hps")
                    for kc in range(8):
                        nc.tensor.matmul(
                            out=ps,
                            lhsT=r32(W["w7g"][:, kc * 512 + mt * 128 : kc * 512 + (mt + 1) * 128]),
                            rhs=r32(gmax[:, kc : kc + 1]),
                            start=(kc == 0), stop=(kc == 7))
                    nc.scalar.activation(out=b7[:, mt : mt + 1], in_=ps, func=AF.Copy)
                nc.vector.tensor_mul(b7, b7, W["s7"])
                nc.vector.tensor_add(b7, b7, W["t7"])

                # h7 = Lrelu(s7*(w7x@cat2) + b7)  [128, 4*N]
                h7 = hd.tile([128, 4 * N], F32, tag="h7")
                for mt in range(4):
                    for nch in range(4):
                        ps = hd_ps.tile([128, 512], F32, tag="hps")
                        sl = bass.ts(nch, 512)
                        nc.tensor.matmul(out=ps, lhsT=r32(W["w7xa"][:, mt * 128 : (mt + 1) * 128]),
                                         rhs=r32(cat[:, sl]), start=True, stop=False)
                        nc.tensor.matmul(out=ps, lhsT=r32(W["w7xb"][:, mt * 128 : (mt + 1) * 128]),
                                         rhs=r32(x3s[:, sl]), start=False, stop=True)
                        nc.scalar.activation(out=h7[:, mt * N + nch * 512 : mt * N + (nch + 1) * 512],
                                             in_=ps, func=AF.Prelu, alpha=0.2,
                                             scale=W["s7"][:, mt : mt + 1],
                                             bias=b7[:, mt : mt + 1])

                # h8 = Lrelu(s8*(w8@h7) + t8) [128, 2*N]
                h8 = hd.tile([128, 2 * N], F32, tag="h8")
                for mt in range(2):
                    for nch in range(4):
                        ps = hd_ps.tile([128, 512], F32, tag="hps")
                        sl = bass.ts(nch, 512)
                        for kc in range(4):
                            nc.tensor.matmul(
                                out=ps,
                                lhsT=r32(W["w8t"][:, kc * 256 + mt * 128 : kc * 256 + (mt + 1) * 128]),
                                rhs=r32(h7[:, kc * N + nch * 512 : kc * N + (nch + 1) * 512]),
                                start=(kc == 0), stop=(kc == 3))
                        nc.scalar.activation(out=h8[:, mt * N + nch * 512 : mt * N + (nch + 1) * 512],
                                             in_=ps, func=AF.Prelu, alpha=0.2,
                                             scale=W["s8"][:, mt : mt + 1],
                                             bias=W["t8"][:, mt : mt + 1])

                # y = sigmoid(w9 @ h8) [1, N]
                yrow = hd.tile([1, N], F32, tag="yrow")
                for nch in range(4):
                    ps = hd_ps.tile([1, 512], F32, tag="hps")
                    sl = bass.ts(nch, 512)
                    for kc in range(2):
                        nc.tensor.matmul(
                            out=ps, lhsT=r32(W["w9t"][:, kc : kc + 1]),
                            rhs=r32(h8[:, kc * N + nch * 512 : kc * N + (nch + 1) * 512]),
                            start=(kc == 0), stop=(kc == 1))
                    nc.scalar.activation(out=yrow[:, sl], in_=ps, func=AF.Sigmoid)
                nc.sync.dma_start(out=y_out[s : s + 1, :], in_=yrow)
            hs_cm.__exit__(None, None, None)
            hd_ps_cm.__exit__(None, None, None)
            hd_cm.__exit__(None, None, None)
        cpool_cm.__exit__(None, None, None)
    split_sem_waits(nc)
    return nc


_NC_CACHE = {}
LAST_RESULT = None  # BassKernelResults of the most recent kernel() call


def kernel(**inputs):
    global LAST_RESULT
    stage = inputs.pop("_stage", 4)
    debug = inputs.pop("_debug", False)
    trace = inputs.pop("_trace", False)
    key = (stage, debug)
    if key not in _NC_CACHE:
        _NC_CACHE[key] = build_nc(stage, debug)
    nc = _NC_CACHE[key]

    w = host_preprocess(inputs)
    x = np.asarray(inputs["x"], np.float32)  # [16, 2, 2048]
    in_maps = []
    for core in range(8):
        m = dict(w) if stage >= 4 else {
            k: v for k, v in w.items()
            if not k.startswith(("w6", "w7", "w8", "w9", "s6", "s7", "s8", "t6", "t7", "t8"))
        }
        m["xpk"] = np.concatenate([x[2 * core], x[2 * core + 1]], 0)  # [4, N]
        in_maps.append(m)
    res = run_bass_kernel_spmd(nc, in_maps, core_ids=list(range(8)), trace=trace)
    LAST_RESULT = res
    outs = [res.results[i]["y"] for i in range(8)]
    if stage >= 4:
        return np.concatenate(outs, 0).astype(np.float32)  # [16, N]
    return np.stack(outs, 0)  # debug: [8, 128, N]

